# revision 62
# baseline (speedup 1.0000x reference)
"""Trainium2 Bass kernel: MultiHeadSelfAttention (B=1, S=4096, D=512, H=8, DK=DV=64)
with fc_out applied twice.

Sharding: 2-way sequence x 4-way head-pair hybrid. Core c = (s, g) with
s = c//4, g = c%4 handles queries [2048s : 2048s+2048] for head pair g
(heads 2g, 2g+1):
  - Wq/Wk/Wv column-sharded by pair: each core projects only its pair's
    K^T/V over the full 4096 keys.
  - fc_out row-sharded: each core computes the partial y^T = W2[pair rows]^T
    @ att^T for its 2048 queries; the HOST sums the 4 pair-partials per
    sequence half and adds the (folded) bias. No collectives anywhere.
  - attention runs as 8 "virtual heads" (4 query chunks of 512 x head lo/hi).

Software pipeline (the key structure): vhead v's scores+exp run in window v,
its attn@V in window v+1. Window 0 emits TWO score streams (vheads 0 and 1)
plus all projections, so the ACT exp chain - the end-to-end critical path -
is never starved afterwards; the attn@V work cascades one window behind its
scores. Schedule:
  W0: sc0+sc1 + qproj + K-proj + V-proj drip        (psum: scores 6 + bg 2)
  W1: sc2+av0   W2: sc3+av1   W3: sc4+av2+fc0       (psum: scores 6 + av 2)
  W4: sc5+av3+av4             W5: sc6+av5+fc1
  W6: sc7+av6+av7(lagged)+fc2
  tail: norm7, fc3, casts, DMA out
All fc matmuls borrow scores-pool rotation slots (their reader is a fast
cast, not an exp, so the exp cadence is kept); the av pool is double-banked
so a window-boundary attn@V never WARs on the previous norm's reads.
Window walls: W0 ~40us PE/DMA-bound (ACT pre-loads 2 vheads of exp), W1-W5
ACT-paced ~15.7us, W4/W6 PE-bound (ACT catches up), tail ~6us. The K0
projection runs in seq-halves on a split kst0 DMA so the first exp lands
at ~18us instead of ~21us.

Layout notes:
  - scores^T tiles [seq_k(128) x seq_q(512)] via lhsT=K^T-pair block,
    rhs=q^T slot. K^T packs head lo on rows 0-63, head hi on 64-127; q^T
    slots zero the complementary rows so K=128 matmuls never trip the PE
    HAM activity monitor (K=64 pins the clock to 1.2 GHz).
  - softmax denominator via a ones-column appended to each head's V (stride
    65): attn@V gives [65, 512] per vhead = output^T rows + exp-sum row.
  - the two fc_out applications are folded on the host (W2 = Wo@Wo,
    b2 = bo@Wo + bo); bias is added on host after the partial sum.
  - output returned TRANSPOSED ([D, 2048] bf16 partial); host sums and
    un-transposes. fc drips one dout-chunk per chunk-slot so each matmul is
    ready when the PE reaches it (wait-queue depth 4, head-of-line blocking).
  - 20 throwaway matmuls lead the PE stream to ramp the clock while the
    first DMAs land.
"""
import sys, functools
sys.path.insert(0, "/opt/trn_rl_repo")
if "/root/.axon_site" not in sys.path:
    sys.path.insert(0, "/root/.axon_site")
import numpy as np
import ml_dtypes

import concourse.bass as bass
import concourse.tile as tile
from concourse import bacc, mybir, masks
from concourse.bass_utils import run_bass_kernel_spmd

NCORES = 8
S, D, H, DK = 4096, 512, 8, 64
SEQW = 2
HPW = 4
CH = S // SEQW    # 2048 queries per core
NV = CH // 512    # 4 query chunks -> 8 virtual heads
VW = 2 * (DK + 1)           # 130: pair v row width incl. ones columns
JT = S // 128               # 32 seq_k tiles
CHUNK = 3                   # j-tiles per exp batch ([128,1536] psum)
NCH = (JT + CHUNK - 1) // CHUNK   # 11 chunks per vhead

F32 = mybir.dt.float32
BF16 = mybir.dt.bfloat16
EXP = mybir.ActivationFunctionType.Exp


def _build_program():
    nc = bacc.Bacc("TRN2", target_bir_lowering=False, debug=False,
                   num_devices=NCORES)

    # all staged inputs are host-prepacked into the exact SBUF tile layouts
    # (partition-major, 4KB contiguous per partition line) so every DMA moves
    # full lines instead of 1KB strided pieces
    xqS = nc.dram_tensor("xqS", [128, 4 * CH], BF16, kind="ExternalInput")
    keysS = nc.dram_tensor("keysS", [128, 8 * 2048], BF16, kind="ExternalInput")
    valsS = nc.dram_tensor("valsS", [128, 8 * 2048], BF16, kind="ExternalInput")
    Wq = nc.dram_tensor("Wq", [128, 512], BF16, kind="ExternalInput")
    Wk = nc.dram_tensor("Wk", [128, 512], BF16, kind="ExternalInput")
    Wv = nc.dram_tensor("Wv", [128, 512], BF16, kind="ExternalInput")
    W2 = nc.dram_tensor("W2", [128, D], BF16, kind="ExternalInput")
    yT = nc.dram_tensor("yT", [D, CH], BF16, kind="ExternalOutput")
    yT_d = yT.ap().rearrange("(m p) f -> p m f", m=4, p=128)

    with tile.TileContext(nc) as tc:
        with tc.tile_pool(name="persist", bufs=1) as pp, \
             tc.tile_pool(name="kv", bufs=1) as kvp, \
             tc.tile_pool(name="pt", bufs=23) as ptp, \
             tc.tile_pool(name="rc", bufs=2) as rcp:

            W2_sb = pp.tile([128, 512], BF16, tag="w2")
            Wk_sb = pp.tile([128, 512], BF16, tag="wk")
            Wv_sb = pp.tile([128, 512], BF16, tag="wv")
            ones64b = pp.tile([1, 64], BF16, tag="on")
            warm_sb = pp.tile([128, 256], BF16, tag="warm")
            o2p = [pp.tile([128, 512], BF16, tag=f"o2{m}", name=f"o2_{m}")
                   for m in range(4)]
            o2all = pp.tile([128, 2048], BF16, tag="o2a")
            qTz_sb = pp.tile([128, 2 * NV * 512], BF16, tag="qt")
            attTp = [pp.tile([128, 512], BF16, tag=f"att{p}", name=f"attT{p}")
                     for p in range(4)]
            KTp = kvp.tile([128, S], BF16, tag="kt")
            V_sb = kvp.tile([128, JT * VW], BF16, tag="v")

            nc.vector.memset(warm_sb[:], 0.0)
            nc.vector.memset(qTz_sb[:], 0.0)
            nc.vector.memset(ones64b[:], 1.0)
            nc.gpsimd.memset(
                V_sb[:].rearrange("p (j h x) -> p j h x", j=JT, h=2, x=DK + 1)
                [:, :, :, DK:DK + 1], 1.0)

            # scores psum pool spans all windows (6 banks)
            scp_cm = tc.tile_pool(name="ps_sc", bufs=2, space="PSUM")
            scp = scp_cm.__enter__()

            # ---- window 0 scope: staging + projection psums ----
            ksp_cm = tc.tile_pool(name="kstage", bufs=1)
            ksp = ksp_cm.__enter__()
            xin_cm = tc.tile_pool(name="xin", bufs=1)
            xp = xin_cm.__enter__()
            bgp_cm = tc.tile_pool(name="ps_bg", bufs=2, space="PSUM")
            bgp = bgp_cm.__enter__()

            Wq_sb = xp.tile([128, 512], BF16, tag="wq")
            xqT_sb = xp.tile([128, 4 * CH], BF16, tag="xq")
            # one tile per 512-seq chunk so each projection group depends on
            # exactly its own chunk's DMA (fused 1MB transfers measured worse:
            # coarser waits outweigh fewer descriptors)
            vst = [xp.tile([128, 2048], BF16, tag=f"vs{c}", name=f"vst{c}")
                   for c in range(8)]
            kst0 = [ksp.tile([128, 2048], BF16, tag=f"ks{c}", name=f"kst{c}")
                    for c in range(8)]

            ks_ = keysS.ap().rearrange("p (c s) -> p c s", c=8)
            vs_ = valsS.ap().rearrange("p (c s) -> p c s", c=8)
            xq_q = xqS.ap().rearrange("p (q s) -> p q s", q=4)
            xqd_q = xqT_sb[:].rearrange("p (q s) -> p q s", q=4)
            # single queue, strictly in consumption order (concurrent pulls on
            # other queues measurably steal early HBM bandwidth): minimal
            # first-compute set leads; qp2/qp3 inputs ride late since their
            # vheads only run from W3
            nc.sync.dma_start(Wq_sb[:], Wq.ap())
            nc.sync.dma_start(xqd_q[:, 0, :], xq_q[:, 0, :])
            nc.sync.dma_start(Wk_sb[:], Wk.ap())
            k0_d = kst0[0][:].rearrange("p (k s) -> p k s", k=4)
            k0_s = ks_[:, 0, :].rearrange("p (k s) -> p k s", k=4)
            nc.sync.dma_start(k0_d[:, :, 0:256], k0_s[:, :, 0:256])
            nc.sync.dma_start(k0_d[:, :, 256:512], k0_s[:, :, 256:512])
            nc.sync.dma_start(xqd_q[:, 1, :], xq_q[:, 1, :])
            nc.sync.dma_start(Wv_sb[:], Wv.ap())
            order = [(0, 1), (1, 0), (0, 2), (1, 1), (0, 3), (1, 2), (2, 2),
                     (2, 3), (0, 4), (1, 3), (0, 5), (1, 4), (0, 6), (1, 5),
                     (0, 7), (1, 6), (1, 7)]
            for which, ci in order:
                if which == 0:
                    nc.sync.dma_start(kst0[ci][:], ks_[:, ci, :])
                elif which == 1:
                    nc.sync.dma_start(vst[ci][:], vs_[:, ci, :])
                else:
                    nc.sync.dma_start(xqd_q[:, ci, :], xq_q[:, ci, :])
            nc.sync.dma_start(W2_sb[:], W2.ap())

            # PE p-state warmup through the bg pool (throwaway matmuls)
            for i in range(20):
                wp = bgp.tile([128, 512], F32, tag="bg", name=f"warm{i}")
                nc.tensor.matmul(wp[0:64, 0:256], lhsT=warm_sb[:, 0:64],
                                 rhs=warm_sb[:, 0:256], start=True, stop=True)

            def q_proj(qc, pool):
                # xq layout is [p, (qc, k, s)] so each 512-query quarter is a
                # single contiguous 4KB-line DMA
                ps = pool.tile([128, 512], F32, tag="bg", name=f"qp{qc}")
                for k in range(4):
                    nc.tensor.matmul(
                        ps[:], lhsT=Wq_sb[:, 128 * k:128 * k + 128],
                        rhs=xqT_sb[:, 2048 * qc + 512 * k:2048 * qc + 512 * k + 512],
                        start=(k == 0), stop=(k == 3))
                nc.vector.tensor_copy(
                    qTz_sb[0:64, 512 * (2 * qc):512 * (2 * qc) + 512], ps[0:64, :])
                nc.vector.tensor_copy(
                    qTz_sb[64:128, 512 * (2 * qc + 1):512 * (2 * qc + 1) + 512],
                    ps[64:128, :])

            def v_proj_group(j, pool=None, tag="bg"):
                pool = pool if pool is not None else bgp
                shape = [128, 512] if tag == "bg" else [128, 1536]
                ps = pool.tile(shape, F32, tag=tag, name=f"vp{j}")
                vt, jj = vst[j // 4], j % 4
                for k in range(4):
                    nc.tensor.matmul(
                        ps[0:128, 0:128],
                        lhsT=vt[:, 512 * k + 128 * jj:512 * k + 128 * jj + 128],
                        rhs=Wv_sb[:, 128 * k:128 * k + 128],
                        start=(k == 0), stop=(k == 3))
                dst = V_sb[:, VW * j:VW * j + VW].rearrange(
                    "p (h x) -> p h x", h=2, x=DK + 1)[:, :, 0:DK]
                nc.vector.tensor_copy(
                    dst, ps[0:128, 0:128].rearrange("p (h x) -> p h x", h=2, x=DK))

            def k_proj_group(sc):
                ps = bgp.tile([128, 512], F32, tag="bg", name=f"kp{sc}")
                for k in range(4):
                    nc.tensor.matmul(
                        ps[:], lhsT=Wk_sb[:, 128 * k:128 * k + 128],
                        rhs=kst0[sc][:, 512 * k:512 * k + 512],
                        start=(k == 0), stop=(k == 3))
                nc.vector.tensor_copy(KTp[:, 512 * sc:512 * sc + 512], ps[:])

            pts = {v: [] for v in range(2 * NV)}   # per-vhead pt tiles

            def chunk_js(c):
                return list(range(CHUNK * c, min(CHUNK * c + CHUNK, JT)))

            def scores_chunk(v, c, split_exp=False):
                js = chunk_js(c)
                ps = scp.tile([128, 512 * CHUNK], F32, tag="sc",
                              name=f"sc{v}_{c}")
                pt = ptp.tile([128, 512 * CHUNK], BF16, tag="pt",
                              name=f"pt{v}_{c}")
                q_ap = qTz_sb[:, 512 * v:512 * v + 512]
                for i, j in enumerate(js):
                    nc.tensor.matmul(
                        ps[:, 512 * i:512 * i + 512],
                        lhsT=KTp[:, 128 * j:128 * j + 128],
                        rhs=q_ap, start=True, stop=True)
                if split_exp:
                    # per-512 exps so the (region-tracked) attn@V consumer can
                    # start on the first slice — used for the final chunk on
                    # the tail-critical vhead
                    for i in range(len(js)):
                        nc.scalar.activation(pt[:, 512 * i:512 * i + 512],
                                             ps[:, 512 * i:512 * i + 512],
                                             EXP, scale=0.125)
                else:
                    w = 512 * len(js)
                    nc.scalar.activation(pt[:, 0:w], ps[:, 0:w], EXP, scale=0.125)
                pts[v].append(pt)

            def attn_batch(v, av, c):
                voff = 65 * (v % 2)
                for i, j in enumerate(chunk_js(c)):
                    nc.tensor.matmul(
                        av[0:65, :],
                        lhsT=V_sb[:, VW * j + voff:VW * j + voff + 65],
                        rhs=pts[v][c][:, 512 * i:512 * i + 512],
                        start=(j == 0), stop=(j == JT - 1))

            def norm_std(v, av):
                qc, hl = v // 2, v % 2
                att_dst = attTp[qc][64 * hl:64 * hl + 64, :]
                avc = rcp.tile([65, 512], F32, tag="avc", name=f"avc{v}")
                rbc = rcp.tile([64, 512], F32, tag="rb", name=f"rb{v}")
                rtmp = rcp.tile([1, 512], F32, tag="rt", name=f"rt{v}")
                nc.vector.tensor_copy(avc[:], av[0:65, :])
                nc.vector.tensor_copy(rtmp[:], av[64:65, :])
                rb2 = rcp.tile([64, 512], F32, tag="rb2", name=f"rb2{v}")
                nc.gpsimd.partition_broadcast(rbc[:], rtmp[:])
                nc.vector.reciprocal_approx_fast(out=rb2[:], in_=rbc[:])
                nc.vector.tensor_mul(att_dst, avc[0:64, :], rb2[:])

            # ---- window 0: sc0 + sc1 + all projections ----
            # qp1 waits the second xq half; emit K0 before it so the K chain
            # isn't head-of-line blocked
            q_proj(0, bgp)
            # K0 in seq-halves: scores chunk 0 needs only KTp[:, 0:384], so
            # the first half unblocks the exp chain ~2us earlier
            for h in range(2):
                ps = bgp.tile([128, 512], F32, tag="bg", name=f"kp0h{h}")
                for k in range(4):
                    nc.tensor.matmul(
                        ps[0:128, 0:256], lhsT=Wk_sb[:, 128 * k:128 * k + 128],
                        rhs=kst0[0][:, 512 * k + 256 * h:512 * k + 256 * h + 256],
                        start=(k == 0), stop=(k == 3))
                nc.vector.tensor_copy(
                    KTp[:, 256 * h:256 * h + 256], ps[0:128, 0:256])
            q_proj(1, bgp)
            # scores lead each chunk so the exp chain (the end-to-end critical
            # path) starts at ~8us instead of queueing behind V-proj groups
            # that stall on vst DMA arrival
            for c in range(NCH):
                # both scores chunks lead: the K drip waits on a later kst
                # arrival and would head-of-line block sc1 behind it
                scores_chunk(0, c)
                scores_chunk(1, c)
                if c + 1 < 8:
                    k_proj_group(c + 1)
                if c == 4:
                    q_proj(2, bgp)
                if c == 6:
                    q_proj(3, bgp)
                for j in (3 * c, 3 * c + 1, 3 * c + 2):
                    if j < JT:
                        v_proj_group(j)

            bgp_cm.__exit__(None, None, None)
            xin_cm.__exit__(None, None, None)
            ksp_cm.__exit__(None, None, None)

            # ---- windows 1-6 + tail ----
            with tc.tile_pool(name="ps_av", bufs=2, space="PSUM") as psav:

                def fc_emit(qc, m):
                    # fc tiles borrow scores-pool rotation slots; their reader
                    # is a fast cast (not an exp) so the exp cadence is kept
                    ps = scp.tile([128, 1536], F32, tag="sc",
                                  name=f"fc{qc}_{m}")
                    nc.tensor.matmul(
                        ps[:, 0:512], lhsT=W2_sb[:, 128 * m:128 * m + 128],
                        rhs=attTp[qc][:], start=True, stop=True)
                    nc.vector.tensor_copy(o2p[m][:], ps[:, 0:512])
                    eng = nc.sync if m % 2 == 0 else nc.gpsimd
                    eng.dma_start(yT_d[:, m, 512 * qc:512 * qc + 512], o2p[m][:])

                # (scores_vhead, [av_vheads], fc_qc, lagged_self_av)
                def av_tile(v):
                    return psav.tile([65, 512], F32, tag="av", name=f"av{v}")

                WIN = [
                    (2, [0], None, False),
                    (3, [1], None, False),
                    (4, [2], 0, False),
                    (5, [3, 4], None, False),
                    (6, [5], 1, False),
                    (7, [6], 2, True),
                ]
                for sv, avl, fcqc, lagged in WIN:
                    avts = [(v, av_tile(v)) for v in avl]
                    lag_av = av_tile(sv) if lagged else None
                    # PE-fat windows (2 av streams or the lagged tail window)
                    # aren't ACT-paced: scores lead every chunk there so the
                    # exp chain is never queued behind av batches
                    sc_first_all = len(avl) > 1 or lagged
                    pend = None
                    for c in range(NCH):
                        if c == 0 or sc_first_all:
                            # scores first at window entry: the first av batch
                            # WARs on the previous norm's DVE reads, so give
                            # the DVE a head start before the PE needs it
                            scores_chunk(sv, c, split_exp=(
                                lagged and c == NCH - 1))
                        for v, avt in avts:
                            attn_batch(v, avt, c)
                        if fcqc is not None and 1 <= c <= 4:
                            fc_emit(fcqc, c - 1)
                        if c > 0 and not sc_first_all:
                            scores_chunk(sv, c)
                        if lagged:
                            if pend is not None:
                                attn_batch(sv, lag_av, pend)
                            pend = c
                    for v, avt in avts:
                        norm_std(v, avt)
                    if lagged:
                        attn_batch(sv, lag_av, pend)

                # ---- tail: norm7 (fast), fc3, casts, DMA ----
                v7 = 2 * NV - 1
                att_dst = attTp[3][64:128, :]
                rtmpb = rcp.tile([1, 512], BF16, tag="rt", name="rt7")
                # ACT is idle by the tail; DVE still has the norm6 chain queued
                nc.scalar.copy(rtmpb[:], lag_av[64:65, :])
                rbb = psav.tile([65, 512], F32, tag="av", name="rbb7")
                nc.tensor.matmul(rbb[0:64, :], lhsT=ones64b[:], rhs=rtmpb[:],
                                 start=True, stop=True)
                rb2 = rcp.tile([64, 512], F32, tag="rb2", name="rb27")
                nc.vector.reciprocal_approx_fast(out=rb2[:], in_=rbb[0:64, :])
                nc.vector.tensor_mul(att_dst, lag_av[0:64, :], rb2[:])

                # no gate needed: fcA/fcB read attTp[3] whose hi rows are only
                # written by the final mul, so they cannot run early
                # dummy matmuls keep the PE p-state hot through the ~2.7us
                # norm chain so the fc matmuls run at full clock
                wrmT = scp.tile([128, 1536], F32, tag="sc", name="wrmT")
                for _ in range(8):
                    nc.tensor.matmul(wrmT[0:64, 0:256], lhsT=warm_sb[:, 0:64],
                                     rhs=warm_sb[:, 0:256], start=True, stop=True)
                fcA = scp.tile([128, 1536], F32, tag="sc", name="fcA")
                for m in range(3):
                    nc.tensor.matmul(
                        fcA[:, 512 * m:512 * m + 512],
                        lhsT=W2_sb[:, 128 * m:128 * m + 128],
                        rhs=attTp[3][:], start=True, stop=True)
                fcB = scp.tile([128, 1536], F32, tag="sc", name="fcB")
                nc.tensor.matmul(fcB[:, 0:512], lhsT=W2_sb[:, 384:512],
                                 rhs=attTp[3][:], start=True, stop=True)
                # casts alternate engines in fc-completion order; DMAs all
                # on the sync hw queue - a tail gpsimd software-DMA costs a
                # ~2.4us Pool-engine drain at kernel end
                nc.scalar.copy(o2p[0][:], fcA[:, 0:512])
                nc.vector.tensor_copy(o2p[1][:], fcA[:, 512:1024])
                nc.scalar.copy(o2p[2][:], fcA[:, 1024:1536])
                nc.vector.tensor_copy(o2p[3][:], fcB[:, 0:512])
                for m in range(4):
                    # sync + scalar hw queues in parallel: no competing HBM
                    # traffic at the tail, so dual-queue is safe here
                    eng = nc.sync if m % 2 == 0 else nc.scalar
                    eng.dma_start(yT_d[:, m, 1536:2048], o2p[m][:])

            scp_cm.__exit__(None, None, None)

    nc.compile()
    return nc


@functools.lru_cache(maxsize=1)
def _get_program():
    return _build_program()


def _stage_pds(x, inner):
    # [4*128, n*inner] -> [128, n*4*inner] partition-major staged layout:
    # out[p, n_idx*4*inner + k*inner + s] = x[k*128+p, n_idx*inner+s]
    n = x.shape[1] // inner
    return np.ascontiguousarray(
        x.reshape(4, 128, n, inner).transpose(1, 2, 0, 3).reshape(128, -1)
    ).astype(ml_dtypes.bfloat16)


def _make_in_maps(queries, keys, values, Wq, Wk, Wv, Wo, bo):
    q = np.asarray(queries, np.float32).reshape(S, D)
    kT = np.asarray(keys, np.float32).reshape(S, D).T
    vT = np.asarray(values, np.float32).reshape(S, D).T
    Wq = np.asarray(Wq, np.float32)
    Wk = np.asarray(Wk, np.float32)
    Wv = np.asarray(Wv, np.float32)
    W2 = np.asarray(Wo, np.float64) @ np.asarray(Wo, np.float64)
    keysS = _stage_pds(kT, 512)
    valsS = _stage_pds(vT, 512)
    in_maps = []
    for c in range(NCORES):
        s, g = c // HPW, c % HPW
        in_maps.append({
            "xqS": _stage_pds(q[s * CH:(s + 1) * CH].T, 512),
            "keysS": keysS, "valsS": valsS,
            "Wq": _stage_pds(Wq[:, 128 * g:128 * g + 128], 128),
            "Wk": _stage_pds(Wk[:, 128 * g:128 * g + 128], 128),
            "Wv": _stage_pds(Wv[:, 128 * g:128 * g + 128], 128),
            "W2": np.ascontiguousarray(W2[128 * g:128 * g + 128, :]).astype(ml_dtypes.bfloat16),
        })
    return in_maps


def _fold_bias(Wo, bo):
    Wo64 = np.asarray(Wo, np.float64)
    bo64 = np.asarray(bo, np.float64)
    return (bo64 @ Wo64 + bo64).astype(np.float32)


def _run(in_maps, **kw):
    nc = _get_program()
    return run_bass_kernel_spmd(nc, in_maps, core_ids=list(range(NCORES)), **kw)


def _gather(res, b2):
    halves = []
    for s in range(SEQW):
        acc = res.results[s * HPW]["yT"].T.astype(np.float32).copy()
        for g in range(1, HPW):
            acc += res.results[s * HPW + g]["yT"].T.astype(np.float32)
        halves.append(acc + b2)
    return np.concatenate(halves, axis=0).reshape(1, S, D)


def kernel(queries, keys, values, Wq, Wk, Wv, Wo, bo):
    res = _run(_make_in_maps(queries, keys, values, Wq, Wk, Wv, Wo, bo))
    return _gather(res, _fold_bias(Wo, bo))


def run_traced(queries, keys, values, Wq, Wk, Wv, Wo, bo):
    """Like kernel() but with NTFF profiling; returns (output, BassKernelResults)."""
    import types
    import trn_agent_boot.trn_boot as _tb
    from concourse import bass_utils
    hook = _tb._ntff_profile_via_ctypes("/opt/axon/libaxon_pjrt.so")
    mod = types.ModuleType("antenv.axon_hooks")
    mod.get_axon_ntff_profile_hook = lambda: hook
    sys.modules["antenv.axon_hooks"] = mod
    bass_utils.upload_artifacts = lambda tmpdir: tmpdir
    res = _run(_make_in_maps(queries, keys, values, Wq, Wk, Wv, Wo, bo), trace=True)
    return _gather(res, _fold_bias(Wo, bo)), res


# revision 63
# speedup vs baseline: 1.1730x; 1.1730x over previous
"""Trainium2 Bass kernel: MultiHeadSelfAttention (B=1, S=4096, D=512, H=8, DK=DV=64)
with fc_out applied twice.

Sharding: 2-way sequence x 4-way head-pair hybrid. Core c = (s, g) with
s = c//4, g = c%4 handles queries [2048s : 2048s+2048] for head pair g
(heads 2g, 2g+1):
  - Wq/Wk/Wv column-sharded by pair: each core projects only its pair's
    K^T/V over the full 4096 keys.
  - fc_out row-sharded: each core computes the partial y^T = W2[pair rows]^T
    @ att^T for its 2048 queries; the HOST sums the 4 pair-partials per
    sequence half and adds the (folded) bias. No collectives anywhere.
  - attention runs as 8 "virtual heads" (4 query chunks of 512 x head lo/hi).

Software pipeline (the key structure): vhead v's scores+exp run in window v,
its attn@V in window v+1. Window 0 emits TWO score streams (vheads 0 and 1)
plus all projections, so the ACT exp chain - the end-to-end critical path -
is never starved afterwards; the attn@V work cascades one window behind its
scores. Schedule:
  W0: sc0+sc1 + qproj + K-proj + V-proj drip        (psum: scores 6 + bg 2)
  W1: sc2+av0   W2: sc3+av1   W3: sc4+av2+fc0       (psum: scores 6 + av 2)
  W4: sc5+av3+av4             W5: sc6+av5+fc1
  W6: sc7+av6+av7(lagged)+fc2
  tail: norm7, fc3, casts, DMA out
All fc matmuls borrow scores-pool rotation slots (their reader is a fast
cast, not an exp, so the exp cadence is kept); the av pool is double-banked
so a window-boundary attn@V never WARs on the previous norm's reads.
Window walls: W0 ~40us PE/DMA-bound (ACT pre-loads 2 vheads of exp), W1-W5
ACT-paced ~15.7us, W4/W6 PE-bound (ACT catches up), tail ~6us. The K0
projection runs in seq-halves on a split kst0 DMA so the first exp lands
at ~18us instead of ~21us.

Layout notes:
  - scores^T tiles [seq_k(128) x seq_q(512)] via lhsT=K^T-pair block,
    rhs=q^T slot. K^T packs head lo on rows 0-63, head hi on 64-127; q^T
    slots zero the complementary rows so K=128 matmuls never trip the PE
    HAM activity monitor (K=64 pins the clock to 1.2 GHz).
  - softmax denominator via a ones-column appended to each head's V (stride
    65): attn@V gives [65, 512] per vhead = output^T rows + exp-sum row.
  - the two fc_out applications are folded on the host (W2 = Wo@Wo,
    b2 = bo@Wo + bo); bias is added on host after the partial sum.
  - output returned TRANSPOSED ([D, 2048] bf16 partial); host sums and
    un-transposes. fc drips one dout-chunk per chunk-slot so each matmul is
    ready when the PE reaches it (wait-queue depth 4, head-of-line blocking).
  - 20 throwaway matmuls lead the PE stream to ramp the clock while the
    first DMAs land.
"""
import sys, functools
sys.path.insert(0, "/opt/trn_rl_repo")
if "/root/.axon_site" not in sys.path:
    sys.path.insert(0, "/root/.axon_site")
import numpy as np
import ml_dtypes

import concourse.bass as bass
import concourse.tile as tile
from concourse import bacc, mybir, masks
from concourse.bass_utils import run_bass_kernel_spmd

NCORES = 8
S, D, H, DK = 4096, 512, 8, 64
SEQW = 2
HPW = 4
CH = S // SEQW    # 2048 queries per core
NV = CH // 512    # 4 query chunks -> 8 virtual heads
VW = 2 * (DK + 1)           # 130: pair v row width incl. ones columns
JT = S // 128               # 32 seq_k tiles
CHUNK = 3                   # j-tiles per exp batch ([128,1536] psum)
NCH = (JT + CHUNK - 1) // CHUNK   # 11 chunks per vhead

F32 = mybir.dt.float32
BF16 = mybir.dt.bfloat16
EXP = mybir.ActivationFunctionType.Exp


def _build_program():
    nc = bacc.Bacc("TRN2", target_bir_lowering=False, debug=False,
                   num_devices=NCORES)

    # all staged inputs are host-prepacked into the exact SBUF tile layouts
    # (partition-major, 4KB contiguous per partition line) so every DMA moves
    # full lines instead of 1KB strided pieces
    xqS = nc.dram_tensor("xqS", [128, 4 * CH], BF16, kind="ExternalInput")
    keysS = nc.dram_tensor("keysS", [128, 8 * 2048], BF16, kind="ExternalInput")
    valsS = nc.dram_tensor("valsS", [128, 8 * 2048], BF16, kind="ExternalInput")
    Wq = nc.dram_tensor("Wq", [128, 512], BF16, kind="ExternalInput")
    Wk = nc.dram_tensor("Wk", [128, 512], BF16, kind="ExternalInput")
    Wv = nc.dram_tensor("Wv", [128, 512], BF16, kind="ExternalInput")
    W2 = nc.dram_tensor("W2", [128, D], BF16, kind="ExternalInput")
    yT = nc.dram_tensor("yT", [D, CH], BF16, kind="ExternalOutput")
    yT_d = yT.ap().rearrange("(m p) f -> p m f", m=4, p=128)

    with tile.TileContext(nc) as tc:
        with tc.tile_pool(name="persist", bufs=1) as pp, \
             tc.tile_pool(name="kv", bufs=1) as kvp, \
             tc.tile_pool(name="pt", bufs=23) as ptp, \
             tc.tile_pool(name="rc", bufs=2) as rcp:

            W2_sb = pp.tile([128, 512], BF16, tag="w2")
            Wk_sb = pp.tile([128, 512], BF16, tag="wk")
            Wv_sb = pp.tile([128, 512], BF16, tag="wv")
            ones64b = pp.tile([1, 64], BF16, tag="on")
            warm_sb = pp.tile([128, 256], BF16, tag="warm")
            o2p = [pp.tile([128, 512], BF16, tag=f"o2{m}", name=f"o2_{m}")
                   for m in range(4)]
            o2all = pp.tile([128, 2048], BF16, tag="o2a")
            qTz_sb = pp.tile([128, 2 * NV * 512], BF16, tag="qt")
            attTp = [pp.tile([128, 512], BF16, tag=f"att{p}", name=f"attT{p}")
                     for p in range(4)]
            KTp = kvp.tile([128, S], BF16, tag="kt")
            V_sb = kvp.tile([128, JT * VW], BF16, tag="v")

            nc.vector.memset(warm_sb[:], 0.0)
            nc.vector.memset(qTz_sb[:], 0.0)
            nc.vector.memset(ones64b[:], 1.0)
            nc.gpsimd.memset(
                V_sb[:].rearrange("p (j h x) -> p j h x", j=JT, h=2, x=DK + 1)
                [:, :, :, DK:DK + 1], 1.0)

            # scores psum pool spans all windows (6 banks)
            scp_cm = tc.tile_pool(name="ps_sc", bufs=2, space="PSUM")
            scp = scp_cm.__enter__()

            # ---- window 0 scope: staging + projection psums ----
            ksp_cm = tc.tile_pool(name="kstage", bufs=1)
            ksp = ksp_cm.__enter__()
            xin_cm = tc.tile_pool(name="xin", bufs=1)
            xp = xin_cm.__enter__()
            bgp_cm = tc.tile_pool(name="ps_bg", bufs=2, space="PSUM")
            bgp = bgp_cm.__enter__()

            Wq_sb = xp.tile([128, 512], BF16, tag="wq")
            xqT_sb = xp.tile([128, 4 * CH], BF16, tag="xq")
            # one tile per 512-seq chunk so each projection group depends on
            # exactly its own chunk's DMA (fused 1MB transfers measured worse:
            # coarser waits outweigh fewer descriptors)
            vst = [xp.tile([128, 2048], BF16, tag=f"vs{c}", name=f"vst{c}")
                   for c in range(8)]
            kst0 = [ksp.tile([128, 2048], BF16, tag=f"ks{c}", name=f"kst{c}")
                    for c in range(8)]

            ks_ = keysS.ap().rearrange("p (c s) -> p c s", c=8)
            vs_ = valsS.ap().rearrange("p (c s) -> p c s", c=8)
            xq_q = xqS.ap().rearrange("p (q s) -> p q s", q=4)
            xqd_q = xqT_sb[:].rearrange("p (q s) -> p q s", q=4)
            # single queue, strictly in consumption order (concurrent pulls on
            # other queues measurably steal early HBM bandwidth): minimal
            # first-compute set leads; qp2/qp3 inputs ride late since their
            # vheads only run from W3
            nc.sync.dma_start(Wq_sb[:], Wq.ap())
            nc.sync.dma_start(xqd_q[:, 0, :], xq_q[:, 0, :])
            nc.sync.dma_start(Wk_sb[:], Wk.ap())
            k0_d = kst0[0][:].rearrange("p (k s) -> p k s", k=4)
            k0_s = ks_[:, 0, :].rearrange("p (k s) -> p k s", k=4)
            nc.sync.dma_start(k0_d[:, :, 0:256], k0_s[:, :, 0:256])
            nc.sync.dma_start(k0_d[:, :, 256:512], k0_s[:, :, 256:512])
            nc.sync.dma_start(xqd_q[:, 1, :], xq_q[:, 1, :])
            nc.sync.dma_start(Wv_sb[:], Wv.ap())
            order = [(0, 1), (1, 0), (0, 2), (1, 1), (0, 3), (1, 2), (2, 2),
                     (2, 3), (0, 4), (1, 3), (0, 5), (1, 4), (0, 6), (1, 5),
                     (0, 7), (1, 6), (1, 7)]
            for which, ci in order:
                if which == 0:
                    nc.sync.dma_start(kst0[ci][:], ks_[:, ci, :])
                elif which == 1:
                    nc.sync.dma_start(vst[ci][:], vs_[:, ci, :])
                else:
                    nc.sync.dma_start(xqd_q[:, ci, :], xq_q[:, ci, :])
            nc.sync.dma_start(W2_sb[:], W2.ap())

            # PE p-state warmup through the bg pool (throwaway matmuls)
            for i in range(20):
                wp = bgp.tile([128, 512], F32, tag="bg", name=f"warm{i}")
                nc.tensor.matmul(wp[0:64, 0:256], lhsT=warm_sb[:, 0:64],
                                 rhs=warm_sb[:, 0:256], start=True, stop=True)

            def q_proj(qc, pool):
                # xq layout is [p, (qc, k, s)] so each 512-query quarter is a
                # single contiguous 4KB-line DMA
                ps = pool.tile([128, 512], F32, tag="bg", name=f"qp{qc}")
                for k in range(4):
                    nc.tensor.matmul(
                        ps[:], lhsT=Wq_sb[:, 128 * k:128 * k + 128],
                        rhs=xqT_sb[:, 2048 * qc + 512 * k:2048 * qc + 512 * k + 512],
                        start=(k == 0), stop=(k == 3))
                nc.vector.tensor_copy(
                    qTz_sb[0:64, 512 * (2 * qc):512 * (2 * qc) + 512], ps[0:64, :])
                nc.vector.tensor_copy(
                    qTz_sb[64:128, 512 * (2 * qc + 1):512 * (2 * qc + 1) + 512],
                    ps[64:128, :])

            def v_proj_group(j, pool=None, tag="bg"):
                pool = pool if pool is not None else bgp
                shape = [128, 512] if tag == "bg" else [128, 1536]
                ps = pool.tile(shape, F32, tag=tag, name=f"vp{j}")
                vt, jj = vst[j // 4], j % 4
                for k in range(4):
                    nc.tensor.matmul(
                        ps[0:128, 0:128],
                        lhsT=vt[:, 512 * k + 128 * jj:512 * k + 128 * jj + 128],
                        rhs=Wv_sb[:, 128 * k:128 * k + 128],
                        start=(k == 0), stop=(k == 3))
                dst = V_sb[:, VW * j:VW * j + VW].rearrange(
                    "p (h x) -> p h x", h=2, x=DK + 1)[:, :, 0:DK]
                nc.vector.tensor_copy(
                    dst, ps[0:128, 0:128].rearrange("p (h x) -> p h x", h=2, x=DK))

            def k_proj_group(sc):
                ps = bgp.tile([128, 512], F32, tag="bg", name=f"kp{sc}")
                for k in range(4):
                    nc.tensor.matmul(
                        ps[:], lhsT=Wk_sb[:, 128 * k:128 * k + 128],
                        rhs=kst0[sc][:, 512 * k:512 * k + 512],
                        start=(k == 0), stop=(k == 3))
                nc.vector.tensor_copy(KTp[:, 512 * sc:512 * sc + 512], ps[:])

            pts = {v: [] for v in range(2 * NV)}   # per-vhead pt tiles

            def chunk_js(c):
                return list(range(CHUNK * c, min(CHUNK * c + CHUNK, JT)))

            def scores_chunk(v, c, split_exp=False):
                js = chunk_js(c)
                ps = scp.tile([128, 512 * CHUNK], F32, tag="sc",
                              name=f"sc{v}_{c}")
                pt = ptp.tile([128, 512 * CHUNK], BF16, tag="pt",
                              name=f"pt{v}_{c}")
                q_ap = qTz_sb[:, 512 * v:512 * v + 512]
                for i, j in enumerate(js):
                    nc.tensor.matmul(
                        ps[:, 512 * i:512 * i + 512],
                        lhsT=KTp[:, 128 * j:128 * j + 128],
                        rhs=q_ap, start=True, stop=True)
                if split_exp:
                    # per-512 exps so the (region-tracked) attn@V consumer can
                    # start on the first slice — used for the final chunk on
                    # the tail-critical vhead
                    for i in range(len(js)):
                        nc.scalar.activation(pt[:, 512 * i:512 * i + 512],
                                             ps[:, 512 * i:512 * i + 512],
                                             EXP, scale=0.125)
                else:
                    w = 512 * len(js)
                    nc.scalar.activation(pt[:, 0:w], ps[:, 0:w], EXP, scale=0.125)
                pts[v].append(pt)

            def attn_batch(v, av, c):
                voff = 65 * (v % 2)
                for i, j in enumerate(chunk_js(c)):
                    nc.tensor.matmul(
                        av[0:65, :],
                        lhsT=V_sb[:, VW * j + voff:VW * j + voff + 65],
                        rhs=pts[v][c][:, 512 * i:512 * i + 512],
                        start=(j == 0), stop=(j == JT - 1))

            def norm_std(v, av):
                qc, hl = v // 2, v % 2
                att_dst = attTp[qc][64 * hl:64 * hl + 64, :]
                avc = rcp.tile([65, 512], F32, tag="avc", name=f"avc{v}")
                rbc = rcp.tile([64, 512], F32, tag="rb", name=f"rb{v}")
                rtmp = rcp.tile([1, 512], F32, tag="rt", name=f"rt{v}")
                nc.vector.tensor_copy(avc[:], av[0:65, :])
                nc.vector.tensor_copy(rtmp[:], av[64:65, :])
                rb2 = rcp.tile([64, 512], F32, tag="rb2", name=f"rb2{v}")
                nc.gpsimd.partition_broadcast(rbc[:], rtmp[:])
                nc.vector.reciprocal_approx_fast(out=rb2[:], in_=rbc[:])
                nc.vector.tensor_mul(att_dst, avc[0:64, :], rb2[:])

            # ---- window 0: sc0 + sc1 + all projections ----
            # qp1 waits the second xq half; emit K0 before it so the K chain
            # isn't head-of-line blocked
            q_proj(0, bgp)
            # K0 in seq-halves: scores chunk 0 needs only KTp[:, 0:384], so
            # the first half unblocks the exp chain ~2us earlier
            for h in range(2):
                ps = bgp.tile([128, 512], F32, tag="bg", name=f"kp0h{h}")
                for k in range(4):
                    nc.tensor.matmul(
                        ps[0:128, 0:256], lhsT=Wk_sb[:, 128 * k:128 * k + 128],
                        rhs=kst0[0][:, 512 * k + 256 * h:512 * k + 256 * h + 256],
                        start=(k == 0), stop=(k == 3))
                nc.vector.tensor_copy(
                    KTp[:, 256 * h:256 * h + 256], ps[0:128, 0:256])
            q_proj(1, bgp)
            # scores lead each chunk so the exp chain (the end-to-end critical
            # path) starts at ~8us instead of queueing behind V-proj groups
            # that stall on vst DMA arrival
            for c in range(NCH):
                scores_chunk(0, c)
                if c + 1 < 8:
                    k_proj_group(c + 1)
                scores_chunk(1, c)
                if c == 4:
                    q_proj(2, bgp)
                if c == 6:
                    q_proj(3, bgp)
                for j in (3 * c, 3 * c + 1, 3 * c + 2):
                    if j < JT:
                        v_proj_group(j)

            bgp_cm.__exit__(None, None, None)
            xin_cm.__exit__(None, None, None)
            ksp_cm.__exit__(None, None, None)

            # ---- windows 1-6 + tail ----
            with tc.tile_pool(name="ps_av", bufs=2, space="PSUM") as psav:

                def fc_emit(qc, m):
                    # fc tiles borrow scores-pool rotation slots; their reader
                    # is a fast cast (not an exp) so the exp cadence is kept
                    ps = scp.tile([128, 1536], F32, tag="sc",
                                  name=f"fc{qc}_{m}")
                    nc.tensor.matmul(
                        ps[:, 0:512], lhsT=W2_sb[:, 128 * m:128 * m + 128],
                        rhs=attTp[qc][:], start=True, stop=True)
                    nc.vector.tensor_copy(o2p[m][:], ps[:, 0:512])
                    eng = nc.sync if m % 2 == 0 else nc.gpsimd
                    eng.dma_start(yT_d[:, m, 512 * qc:512 * qc + 512], o2p[m][:])

                # (scores_vhead, [av_vheads], fc_qc, lagged_self_av)
                def av_tile(v):
                    return psav.tile([65, 512], F32, tag="av", name=f"av{v}")

                WIN = [
                    (2, [0], None, False),
                    (3, [1], None, False),
                    (4, [2], 0, False),
                    (5, [3, 4], None, False),
                    (6, [5], 1, False),
                    (7, [6], 2, True),
                ]
                for sv, avl, fcqc, lagged in WIN:
                    avts = [(v, av_tile(v)) for v in avl]
                    lag_av = av_tile(sv) if lagged else None
                    # PE-fat windows (2 av streams or the lagged tail window)
                    # aren't ACT-paced: scores lead every chunk there so the
                    # exp chain is never queued behind av batches
                    sc_first_all = len(avl) > 1 or lagged
                    pend = None
                    for c in range(NCH):
                        if c == 0 or sc_first_all:
                            # scores first at window entry: the first av batch
                            # WARs on the previous norm's DVE reads, so give
                            # the DVE a head start before the PE needs it
                            scores_chunk(sv, c, split_exp=(
                                lagged and c == NCH - 1))
                        for v, avt in avts:
                            attn_batch(v, avt, c)
                        if fcqc is not None and 1 <= c <= 4:
                            fc_emit(fcqc, c - 1)
                        if c > 0 and not sc_first_all:
                            scores_chunk(sv, c)
                        if lagged:
                            if pend is not None:
                                attn_batch(sv, lag_av, pend)
                            pend = c
                    for v, avt in avts:
                        norm_std(v, avt)
                    if lagged:
                        attn_batch(sv, lag_av, pend)

                # ---- tail: norm7 (fast), fc3, casts, DMA ----
                v7 = 2 * NV - 1
                att_dst = attTp[3][64:128, :]
                rtmpb = rcp.tile([1, 512], BF16, tag="rt", name="rt7")
                # ACT is idle by the tail; DVE still has the norm6 chain queued
                nc.scalar.copy(rtmpb[:], lag_av[64:65, :])
                rbb = psav.tile([65, 512], F32, tag="av", name="rbb7")
                nc.tensor.matmul(rbb[0:64, :], lhsT=ones64b[:], rhs=rtmpb[:],
                                 start=True, stop=True)
                rb2 = rcp.tile([64, 512], F32, tag="rb2", name="rb27")
                nc.vector.reciprocal_approx_fast(out=rb2[:], in_=rbb[0:64, :])
                nc.vector.tensor_mul(att_dst, lag_av[0:64, :], rb2[:])

                # no gate needed: fcA/fcB read attTp[3] whose hi rows are only
                # written by the final mul, so they cannot run early
                # dummy matmuls keep the PE p-state hot through the ~2.7us
                # norm chain so the fc matmuls run at full clock
                wrmT = scp.tile([128, 1536], F32, tag="sc", name="wrmT")
                for _ in range(8):
                    nc.tensor.matmul(wrmT[0:64, 0:256], lhsT=warm_sb[:, 0:64],
                                     rhs=warm_sb[:, 0:256], start=True, stop=True)
                fcA = scp.tile([128, 1536], F32, tag="sc", name="fcA")
                for m in range(3):
                    nc.tensor.matmul(
                        fcA[:, 512 * m:512 * m + 512],
                        lhsT=W2_sb[:, 128 * m:128 * m + 128],
                        rhs=attTp[3][:], start=True, stop=True)
                fcB = scp.tile([128, 1536], F32, tag="sc", name="fcB")
                nc.tensor.matmul(fcB[:, 0:512], lhsT=W2_sb[:, 384:512],
                                 rhs=attTp[3][:], start=True, stop=True)
                # casts alternate engines in fc-completion order; DMAs all
                # on the sync hw queue - a tail gpsimd software-DMA costs a
                # ~2.4us Pool-engine drain at kernel end
                nc.scalar.copy(o2p[0][:], fcA[:, 0:512])
                nc.vector.tensor_copy(o2p[1][:], fcA[:, 512:1024])
                nc.scalar.copy(o2p[2][:], fcA[:, 1024:1536])
                nc.vector.tensor_copy(o2p[3][:], fcB[:, 0:512])
                for m in range(4):
                    # sync + scalar hw queues in parallel: no competing HBM
                    # traffic at the tail, so dual-queue is safe here
                    eng = nc.sync if m % 2 == 0 else nc.scalar
                    eng.dma_start(yT_d[:, m, 1536:2048], o2p[m][:])

            scp_cm.__exit__(None, None, None)

    nc.compile()
    return nc


@functools.lru_cache(maxsize=1)
def _get_program():
    return _build_program()


def _stage_pds(x, inner):
    # [4*128, n*inner] -> [128, n*4*inner] partition-major staged layout:
    # out[p, n_idx*4*inner + k*inner + s] = x[k*128+p, n_idx*inner+s]
    n = x.shape[1] // inner
    return np.ascontiguousarray(
        x.reshape(4, 128, n, inner).transpose(1, 2, 0, 3).reshape(128, -1)
    ).astype(ml_dtypes.bfloat16)


def _make_in_maps(queries, keys, values, Wq, Wk, Wv, Wo, bo):
    q = np.asarray(queries, np.float32).reshape(S, D)
    kT = np.asarray(keys, np.float32).reshape(S, D).T
    vT = np.asarray(values, np.float32).reshape(S, D).T
    Wq = np.asarray(Wq, np.float32)
    Wk = np.asarray(Wk, np.float32)
    Wv = np.asarray(Wv, np.float32)
    W2 = np.asarray(Wo, np.float64) @ np.asarray(Wo, np.float64)
    keysS = _stage_pds(kT, 512)
    valsS = _stage_pds(vT, 512)
    in_maps = []
    for c in range(NCORES):
        s, g = c // HPW, c % HPW
        in_maps.append({
            "xqS": _stage_pds(q[s * CH:(s + 1) * CH].T, 512),
            "keysS": keysS, "valsS": valsS,
            "Wq": _stage_pds(Wq[:, 128 * g:128 * g + 128], 128),
            "Wk": _stage_pds(Wk[:, 128 * g:128 * g + 128], 128),
            "Wv": _stage_pds(Wv[:, 128 * g:128 * g + 128], 128),
            "W2": np.ascontiguousarray(W2[128 * g:128 * g + 128, :]).astype(ml_dtypes.bfloat16),
        })
    return in_maps


def _fold_bias(Wo, bo):
    Wo64 = np.asarray(Wo, np.float64)
    bo64 = np.asarray(bo, np.float64)
    return (bo64 @ Wo64 + bo64).astype(np.float32)


def _run(in_maps, **kw):
    nc = _get_program()
    return run_bass_kernel_spmd(nc, in_maps, core_ids=list(range(NCORES)), **kw)


def _gather(res, b2):
    halves = []
    for s in range(SEQW):
        acc = res.results[s * HPW]["yT"].T.astype(np.float32).copy()
        for g in range(1, HPW):
            acc += res.results[s * HPW + g]["yT"].T.astype(np.float32)
        halves.append(acc + b2)
    return np.concatenate(halves, axis=0).reshape(1, S, D)


def kernel(queries, keys, values, Wq, Wk, Wv, Wo, bo):
    res = _run(_make_in_maps(queries, keys, values, Wq, Wk, Wv, Wo, bo))
    return _gather(res, _fold_bias(Wo, bo))


def run_traced(queries, keys, values, Wq, Wk, Wv, Wo, bo):
    """Like kernel() but with NTFF profiling; returns (output, BassKernelResults)."""
    import types
    import trn_agent_boot.trn_boot as _tb
    from concourse import bass_utils
    hook = _tb._ntff_profile_via_ctypes("/opt/axon/libaxon_pjrt.so")
    mod = types.ModuleType("antenv.axon_hooks")
    mod.get_axon_ntff_profile_hook = lambda: hook
    sys.modules["antenv.axon_hooks"] = mod
    bass_utils.upload_artifacts = lambda tmpdir: tmpdir
    res = _run(_make_in_maps(queries, keys, values, Wq, Wk, Wv, Wo, bo), trace=True)
    return _gather(res, _fold_bias(Wo, bo)), res
